# revision 5
# baseline (speedup 1.0000x reference)
"""GNN message-passing kernel v2 for 8 Trainium2 NeuronCores.

Reference:  msg = x[edge_index[1]]; out = segment_sum(msg, edge_index[0], N).

v1 profiling showed the bottleneck is the GpSimd q7 SWDGE ucode at ~10ns per
descriptor, serialized on one engine for BOTH the gather (125k/core) and the
scatter-add (125k/core).  v2 keeps the SWDGE gather but deletes the scatter:
the segment-sum is done on-chip by the TensorEngine.

Layout (per core, dst rows [c*12500, (c+1)*12500)):
  - window w = 128 consecutive output rows; 98 windows (12544 padded rows).
  - Tokens (edges) sorted by dst; within a window grouped by src int16-window
    s (src//32768, 4 of them) so each (w, s) run is one gather call range.
  - Slot capacity per (w, s) = 128*B[w,s] with B = max over cores (SPMD: one
    program for all cores).  Pad slots gather x[s*32768] and carry segid -1.
  - Token j sits at msg[p= j%128, col=j//128, :] (dma_gather layout).
  - A block = one msg column = 128 tokens, all belonging to window w.
    DVE builds A[tok, seg] = (segid[tok] == iota[seg]) as f32 one-hot.
    PE: psum[w%8] (+)= A_block.T @ msg_block, start/stop per window.
  - DVE evacuates psum -> staging; ACT writes staging rows sequentially to
    DRAM (static DMA).  No CCE scatter, no RMW, no zeroing pass.

Pipeline: GpSimd gathers chunk ci+1 while DVE/PE/ACT process chunk ci
(chunk = 8 windows).  A-strip single-buffered, msg+staging double-buffered.
"""

import contextlib
import functools

import numpy as np

import concourse.bacc as bacc
import concourse.bass as bass
import concourse.mybir as mybir
from concourse.bass_utils import run_bass_kernel_spmd

N_NODES = 100000
D = 64
N_CORES = 8
SHARD = N_NODES // N_CORES      # 12500
N_WINDOWS = (SHARD + 127) // 128  # 98
OUT_ROWS = N_WINDOWS * 128      # 12544
SRC_WIN = 32768
N_SRCW = (N_NODES + SRC_WIN - 1) // SRC_WIN  # 4
MAX_CALL = 1024
CHUNK_W = 4                     # windows per chunk

_f32 = mybir.dt.float32
_i16 = mybir.dt.int16



def _layout(B):
    """Chunk-s-major slot layout. Returns (run_off [W,S], s_tot)."""
    caps = B * 128
    run_off = np.zeros((N_WINDOWS, N_SRCW), dtype=np.int64)
    pos = 0
    for w0 in range(0, N_WINDOWS, CHUNK_W):
        w1 = min(w0 + CHUNK_W, N_WINDOWS)
        for s in range(N_SRCW):
            for w in range(w0, w1):
                run_off[w, s] = pos
                pos += int(caps[w, s])
    return run_off, pos

def _host_prep(edge_index):
    dst = np.asarray(edge_index[0]).astype(np.int64)
    src = np.asarray(edge_index[1]).astype(np.int64)

    cores = []
    cnts = np.zeros((N_CORES, N_WINDOWS * N_SRCW), dtype=np.int64)
    for c in range(N_CORES):
        m = (dst >= c * SHARD) & (dst < (c + 1) * SHARD)
        dstc = (dst[m] - c * SHARD).astype(np.int32)
        srcc = src[m].astype(np.int32)
        key = ((dstc >> 7) * N_SRCW + (srcc // SRC_WIN)).astype(np.int64)
        o = np.argsort(key, kind="stable")
        dstc, srcc, key = dstc[o], srcc[o], key[o]
        cnts[c] = np.bincount(key, minlength=N_WINDOWS * N_SRCW)
        cores.append((dstc, srcc, key))

    B = -(-cnts.max(axis=0) // 128).reshape(N_WINDOWS, N_SRCW)
    B[:, 0] = np.maximum(B[:, 0], 1)  # ensure >=1 block per window
    run_off, s_tot = _layout(B)
    run_off = run_off.ravel()

    per_core = []
    for dstc, srcc, key in cores:
        src_loc = np.zeros(s_tot, dtype=np.int16)
        seg = np.full(s_tot, -1.0, dtype=np.float32)
        seg_counts = np.bincount(key, minlength=N_WINDOWS * N_SRCW)
        seg_starts = np.concatenate(([0], np.cumsum(seg_counts)[:-1]))
        slot = run_off[key] + (np.arange(key.size) - seg_starts[key])
        src_loc[slot] = (srcc & (SRC_WIN - 1)).astype(np.int16)
        seg[slot] = (dstc & 127).astype(np.float32)
        per_core.append(
            {
                "srcloc": np.ascontiguousarray(
                    src_loc.reshape(-1, 16).T
                ),  # [16, s_tot/16], idx j at [j%16, j//16]
                "segid": np.ascontiguousarray(
                    seg.reshape(-1, 128).T
                ),  # [128, s_tot/128], slot j at [j%128, j//128]
            }
        )

    cfg = (tuple(map(tuple, B.tolist())), s_tot)
    return cfg, per_core


@functools.lru_cache(maxsize=4)
def _build(cfg):
    Bt, s_tot = cfg
    B = np.asarray(Bt, dtype=np.int64)          # [W, S]
    caps = B * 128
    run_off, s_tot2 = _layout(B)
    assert s_tot2 == s_tot

    # chunk structure
    chunks = []  # (w0, w1)
    for w0 in range(0, N_WINDOWS, CHUNK_W):
        chunks.append((w0, min(w0 + CHUNK_W, N_WINDOWS)))
    nch = len(chunks)

    def chunk_base(ci):
        return int(run_off[chunks[ci][0], 0])

    def chunk_end(ci):
        w1 = chunks[ci][1]
        return int(run_off[w1, 0]) if w1 < N_WINDOWS else s_tot

    maxch = max((chunk_end(ci) - chunk_base(ci)) // 128 for ci in range(nch))

    # gather calls per chunk: list of (abs_off, n)
    calls = [[] for _ in range(nch)]
    for ci, (w0, w1) in enumerate(chunks):
        for s in range(N_SRCW):
            cap = int(caps[w0:w1, s].sum())
            if cap == 0:
                continue
            off0 = int(run_off[w0, s])
            for sub in range(0, cap, MAX_CALL):
                calls[ci].append((off0 + sub, min(MAX_CALL, cap - sub), s))
    calls_through = np.cumsum([len(c) for c in calls]).tolist()
    win_through = [chunks[ci][1] for ci in range(nch)]

    nc = bacc.Bacc(None, num_swdge_queues=2)
    x_t = nc.dram_tensor("x", [N_NODES, D], _f32, kind="ExternalInput")
    src_t = nc.dram_tensor("srcloc", [16, s_tot // 16], _i16, kind="ExternalInput")
    seg_t = nc.dram_tensor("segid", [128, s_tot // 128], _f32, kind="ExternalInput")
    iota_t = nc.dram_tensor("iota", [128, 128], _f32, kind="ExternalInput")
    out_t = nc.dram_tensor("out", [OUT_ROWS, D], _f32, kind="ExternalOutput")

    with (
        contextlib.ExitStack() as _ps,
        nc.sbuf_tensor([128, s_tot // 16], _i16) as src_sb,
        nc.sbuf_tensor([128, s_tot // 128], _f32) as seg_sb,
        nc.sbuf_tensor([128, 128], _f32) as iota_sb,
        nc.sbuf_tensor([128, 2 * maxch, D], _f32) as msg_sb,
        nc.sbuf_tensor([128, maxch, 128], _f32) as a_sb,
        nc.sbuf_tensor([128, 2 * CHUNK_W, D], _f32) as stag_sb,
        nc.semaphore("s_pre") as s_pre,
        nc.semaphore("s_pre2") as s_pre2,
        nc.semaphore("s_g0") as s_g0,
        nc.semaphore("s_g1") as s_g1,
        nc.semaphore("s_a") as s_a,
        nc.semaphore("s_w") as s_w,
        nc.semaphore("s_e") as s_e,
        nc.semaphore("s_out") as s_out,
        nc.Block() as block,
    ):
        psums = [
            _ps.enter_context(nc.psum_tensor(f"psw{_i}", [128, D], _f32))
            for _i in range(CHUNK_W)
        ]

        @block.sync
        def _(e):
            for p0 in range(0, 128, 16):
                e.dma_start(src_sb[p0 : p0 + 16, :], src_t[:]).then_inc(s_pre, 16)
            e.dma_start(seg_sb[:], seg_t[:]).then_inc(s_pre2, 16)
            e.dma_start(iota_sb[:], iota_t[:]).then_inc(s_pre2, 16)

        gsems = (s_g0, s_g1)
        # cumulative gather-call count per parity after each chunk
        gwait = []
        _cum = [0, 0]
        for _ci in range(nch):
            _cum[_ci % 2] += len(calls[_ci])
            gwait.append(16 * _cum[_ci % 2])

        @block.gpsimd
        def _(g):
            cnt_reg = nc.alloc_register(mybir.EngineType.Pool, "cnt")
            g.wait_ge(s_pre, 16 * 8)
            for ci in range(nch):
                if ci >= 2:
                    g.wait_ge(s_w, win_through[ci - 2])  # PE done, msg buf free
                base = chunk_base(ci)
                buf = (ci % 2) * maxch
                for off, n, s in calls[ci]:
                    hi = min((s + 1) * SRC_WIN, N_NODES)
                    lo = buf + (off - base) // 128
                    g.reg_mov(cnt_reg, n)
                    g.dma_gather(
                        msg_sb[:, lo : lo + n // 128, :],
                        x_t[s * SRC_WIN : hi, :],
                        src_sb[:, off // 16 : (off + n) // 16],
                        n,
                        cnt_reg,
                        D,
                        queue_num=0,
                    ).then_inc(gsems[ci % 2], 16)

        @block.vector
        def _(v):
            v.wait_ge(s_pre2, 16 * 2)
            for ci in range(nch):
                # build A strip for chunk ci (buffer reused from ci-1)
                if ci >= 1:
                    v.wait_ge(s_w, win_through[ci - 1])  # PE consumed strip ci-1
                base = chunk_base(ci)
                ncols = (chunk_end(ci) - base) // 128
                for lc in range(ncols):
                    gcol = base // 128 + lc
                    ins = v.tensor_tensor(
                        out=a_sb[:, lc, :],
                        in0=seg_sb[:, gcol : gcol + 1].to_broadcast([128, 128]),
                        in1=iota_sb[:],
                        op=mybir.AluOpType.is_equal,
                    )
                    if lc == ncols - 1:
                        ins.then_inc(s_a, 1)
                # evacuate previous chunk's psum windows
                if ci >= 1:
                    pj = ci - 1
                    if pj >= 2:
                        v.wait_ge(s_out, 16 * (pj - 1))  # stag slot free
                    w0, w1 = chunks[pj]
                    for w in range(w0, w1):
                        v.wait_ge(s_w, w + 1)
                        v.tensor_copy(
                            stag_sb[:, (pj % 2) * CHUNK_W + (w - w0), :],
                            psums[w % CHUNK_W][:],
                        ).then_inc(s_e, 1)
            # final chunk evac
            pj = nch - 1
            if pj >= 2:
                v.wait_ge(s_out, 16 * (pj - 1))
            w0, w1 = chunks[pj]
            for w in range(w0, w1):
                v.wait_ge(s_w, w + 1)
                v.tensor_copy(
                    stag_sb[:, (pj % 2) * CHUNK_W + (w - w0), :],
                    psums[w % CHUNK_W][:],
                ).then_inc(s_e, 1)

        @block.tensor
        def _(t):
            for ci in range(nch):
                t.wait_ge(s_a, ci + 1)
                t.wait_ge(gsems[ci % 2], gwait[ci])
                base = chunk_base(ci)
                buf = (ci % 2) * maxch
                w0, w1 = chunks[ci]
                for w in range(w0, w1):
                    if w >= CHUNK_W:
                        t.wait_ge(s_e, w - CHUNK_W + 1)
                    last_s = max(s for s in range(N_SRCW) if B[w, s] > 0)
                    for s in range(N_SRCW):
                        nb = int(B[w, s])
                        if nb == 0:
                            continue
                        c0 = (int(run_off[w, s]) - base) // 128
                        for j in range(nb):
                            ins = t.matmul(
                                out=psums[w % CHUNK_W][:],
                                lhsT=a_sb[:, c0 + j, :],
                                rhs=msg_sb[:, buf + c0 + j, :],
                                start=(s == 0 and j == 0),
                                stop=(s == last_s and j == nb - 1),
                            )
                            if s == last_s and j == nb - 1:
                                ins.then_inc(s_w, 1)

        @block.scalar
        def _(a):
            for ci in range(nch):
                if ci >= 2:
                    a.wait_ge(s_out, 16 * (ci - 1))
                a.wait_ge(s_e, win_through[ci])
                w0, w1 = chunks[ci]
                nw = w1 - w0
                view = out_t[w0 * 128 : w1 * 128, :].rearrange(
                    "(a p) d -> p a d", p=128
                )
                a.dma_start(
                    view, stag_sb[:, (ci % 2) * CHUNK_W : (ci % 2) * CHUNK_W + nw, :]
                ).then_inc(s_out, 16)
            a.wait_ge(s_out, 16 * nch)

    nc.finalize()
    return nc


def _iota_arr():
    return np.broadcast_to(
        np.arange(128, dtype=np.float32), (128, 128)
    ).copy()


def kernel(x, edge_index):
    x = np.ascontiguousarray(np.asarray(x), dtype=np.float32)
    cfg, per_core = _host_prep(edge_index)
    nc = _build(cfg)
    iota = _iota_arr()
    in_maps = [
        {"x": x, "srcloc": pc["srcloc"], "segid": pc["segid"], "iota": iota}
        for pc in per_core
    ]
    res = run_bass_kernel_spmd(nc, in_maps, list(range(N_CORES)))
    out = np.concatenate([res.results[c]["out"][:SHARD] for c in range(N_CORES)])
    return out.astype(np.float32)
